# revision 25
# baseline (speedup 1.0000x reference)
"""Multi-head attention (B=4, S=2048, H=1024, 16 heads) on 8 Trainium2 NeuronCores.

Sharding: data-parallel over (batch, seq-half) -> 8 fully independent shards, no
collectives. Each core computes attention for 1024 query tokens of one batch
element; K/V are computed redundantly for the full 2048-token sequence of that
batch (+25% projection flops buys zero cross-core communication).

Per-core scheme (feature-major "transposed" layout), software-pipelined so the
ACT engine (exp) starts while the PE is still doing projections:

  emit order: V projection -> [qk(0); for hp: ST/exp(hp); qk(hp+1); PV(hp)]
              -> output projection

  - qT/kT feature-major [H, tokens]; each 128-row chunk = one head pair
  - scores transposed S_T[j,i]; both heads of a pair write ONE [128,1024]
    PSUM tile ([h0 512i | h1 512i]) via tile_position row packing -> the
    paired matmuls issue back-to-back and overlap on the PE (K=64 solo
    streams at half rate; pairing measured 143ns vs 492ns per mm)
  - softmax: one exp per [128,1024] tile on ACT ((N+352)/1.2 ns); no max
    subtraction needed (|s|/8 < 8 << 88); denominators ride along as a 65th
    ones-column of v in the PV matmul
  - PV: stationary = probs chunk, moving = v_ext; all 8 accumulation chains
    (4 ic x 2 heads) of an (hp,ih) live in ONE [128,776] PSUM tile: cols
    0..519 hold the 8 o_ext chains, cols 520..775 take the PE-transposed
    (feature-major) outputs via a bf16 bitcast view -> probs tiles are
    consumed jc-at-a-time (prb pool can be tiny -> SBUF fits the overlap)
  - divide by denominator: reciprocal + per-partition tensor_scalar on DVE
  - output projection accumulates all 16 heads in PSUM (contraction over
    feature chunks = head pairs)
  - biases: bq/bk per-partition adds on the PSUM->SBUF copy (DVE); bv and bo
    folded into one exact host-side fp32 correction (+ bo + bv @ wo.T --
    softmax rows sum to 1, so a v-bias passes through attention additively).
"""

import numpy as np
import ml_dtypes
from contextlib import ExitStack

import concourse.tile as tile
from concourse import bacc, mybir
from concourse.bass_utils import run_bass_kernel_spmd
from concourse.masks import make_identity

B, S, H, NH, HD = 4, 2048, 1024, 16, 64
T = 1024          # query tokens per core
TK = 2048         # key tokens per core
NCORES = 8
FC = H // 128     # 8 feature chunks (= head pairs)
KC = TK // 128    # 16 key-token chunks
F32 = mybir.dt.float32
BF16 = mybir.dt.bfloat16
BFNP = ml_dtypes.bfloat16
Act = mybir.ActivationFunctionType
Alu = mybir.AluOpType

# po tile layout (f32 cols, [128,1024] = 2 PSUM banks; no accumulation chain
# crosses a 2KB zero-region boundary):
#   bank0: chains ic0/ic1/ic2 at 0/130/260 (65 cols per head), tr0 at 390
#   bank1: chain ic3 at 512, tr1/tr2/tr3 at 642/706/770
CHAIN_OFF = [0, 130, 260, 512]
TR_OFF = [390, 642, 706, 770]
POW = 1024

_CACHE = {}


def _emit(ctx: ExitStack, tc, d):
    nc = tc.nc
    xT, wqT, wkT, wvT = d["xT"], d["wqT"], d["wkT"], d["wvT"]
    woT, bqc, bkc, out = d["woT"], d["bqc"], d["bkc"], d["out"]

    pers = ctx.enter_context(tc.tile_pool(name="pers", bufs=1))
    ident = pers.tile([128, 128], BF16, tag="ident")
    make_identity(nc, ident[:])

    kT_t = [pers.tile([128, TK], BF16, tag="kT", bufs=FC, name=f"kT{i}") for i in range(FC)]
    qT_t = [pers.tile([128, T], BF16, tag="qT", bufs=FC, name=f"qT{i}") for i in range(FC)]
    v_t = [pers.tile([128, NH * (HD + 1)], BF16, tag="vsb", bufs=KC, name=f"v{i}") for i in range(KC)]
    oT_t = [pers.tile([128, T], BF16, tag="oT", bufs=FC, name=f"oT{i}") for i in range(FC)]
    bq_sb = pers.tile([128, FC], F32, tag="bq")
    bk_sb = pers.tile([128, FC], F32, tag="bk")
    nc.sync.dma_start(bq_sb[:], bqc[:, :])
    nc.sync.dma_start(bk_sb[:], bkc[:, :])

    # attention working pools, allocated below the ph1 input pools so their
    # slots don't depend on ph1 frees (enables ph1/attention overlap)
    attn = ctx.enter_context(tc.tile_pool(name="attn", bufs=1))
    # PSUM stack: pst(4) + po(2) live for the whole attention; ppool(2) on
    # top, released after the last qk chain; pf(2) then reuses its banks.
    pst = ctx.enter_context(tc.tile_pool(name="pst", bufs=2, space="PSUM"))
    po = ctx.enter_context(tc.tile_pool(name="po", bufs=1, space="PSUM"))

    stack_x = ExitStack()
    stack_qk = ExitStack()
    stack_v = ExitStack()

    pin = stack_x.enter_context(tc.tile_pool(name="pin", bufs=1))
    x_t = [pin.tile([128, TK], BF16, tag="xin", bufs=FC, name=f"x{i}") for i in range(FC)]
    # wq/wk staged in column halves: 8 slots of [128,512] per tensor; the
    # second-half tiles reuse the slots once head pairs 0..3 are projected.
    pqk = stack_qk.enter_context(tc.tile_pool(name="pqk", bufs=1))
    wq_t = {}
    wk_t = {}
    for half in range(2):
        for i in range(FC):
            wq_t[half, i] = pqk.tile([128, 512], BF16, tag="wqin", bufs=FC,
                                     name=f"wq{half}_{i}")
            wk_t[half, i] = pqk.tile([128, 512], BF16, tag="wkin", bufs=FC,
                                     name=f"wk{half}_{i}")
    pv_in = stack_v.enter_context(tc.tile_pool(name="pvin", bufs=1))
    wv_t = [pv_in.tile([128, H], BF16, tag="wvin", bufs=FC, name=f"wv{i}") for i in range(FC)]

    for fc in range(FC):
        # column-split so the first v chains unblock after half the bytes
        nc.sync.dma_start(x_t[fc][:, 0:1024], xT[fc * 128:(fc + 1) * 128, 0:1024])
        nc.sync.dma_start(wv_t[fc][:, 0:512], wvT[fc * 128:(fc + 1) * 128, 0:512])
    for fc in range(FC):
        nc.sync.dma_start(x_t[fc][:, 1024:2048], xT[fc * 128:(fc + 1) * 128, 1024:2048])
        nc.sync.dma_start(wv_t[fc][:, 512:1024], wvT[fc * 128:(fc + 1) * 128, 512:1024])
    for half in range(2):
        for fc in range(FC):
            c0 = half * 512
            nc.sync.dma_start(wq_t[half, fc][:], wqT[fc * 128:(fc + 1) * 128, c0:c0 + 512])
            nc.sync.dma_start(wk_t[half, fc][:], wkT[fc * 128:(fc + 1) * 128, c0:c0 + 512])
    pp = ExitStack()
    ppool = pp.enter_context(tc.tile_pool(name="ppool", bufs=2, space="PSUM"))

    # ---- V projection, token-major ----
    # bv is NOT applied on device: softmax rows sum to 1, so a v-bias passes
    # through attention additively and is folded into the host-side output
    # correction (bv @ wo.T), exactly.
    def emit_v():
        for kc in range(KC):
            nc.gpsimd.memset(v_t[kc][:], 1.0)  # ones cols survive at 65*h+64
            for mh in range(H // 512):
                ps = ppool.tile([128, 512], F32, tag="pp", name=f"psv{kc}_{mh}")
                for c in range(FC):
                    nc.tensor.matmul(
                        ps[:], lhsT=x_t[c][:, kc * 128:(kc + 1) * 128],
                        rhs=wv_t[c][:, mh * 512:(mh + 1) * 512],
                        start=(c == 0), stop=(c == FC - 1))
                # one strided copy scatters the 8 head slices (65-col pitch);
                # runs on ACT, which is otherwise idle during the V phase
                dst = v_t[kc][:, mh * 8 * (HD + 1):(mh + 1) * 8 * (HD + 1)]
                dst = dst.rearrange("p (h c) -> p h c", h=8)[:, :, 0:HD]
                src = ps[:, 0:512].rearrange("p (h c) -> p h c", h=8)
                nc.scalar.copy(dst, src)
        stack_v.close()

    def emit_qk(fc):
        half, cb = fc // 4, (fc % 4) * 128
        for th in range(T // 512):
            ps = ppool.tile([128, 512], F32, tag="pp", name=f"psq{fc}_{th}")
            for c in range(FC):
                nc.tensor.matmul(
                    ps[:], lhsT=wq_t[half, c][:, cb:cb + 128],
                    rhs=x_t[c][:, th * 512:(th + 1) * 512],
                    start=(c == 0), stop=(c == FC - 1))
            nc.vector.tensor_scalar(qT_t[fc][:, th * 512:(th + 1) * 512], ps[:],
                                    bq_sb[:, fc:fc + 1], None, Alu.add)
        for th in range(TK // 512):
            ps = ppool.tile([128, 512], F32, tag="pp", name=f"psk{fc}_{th}")
            for c in range(FC):
                nc.tensor.matmul(
                    ps[:], lhsT=wk_t[half, c][:, cb:cb + 128],
                    rhs=x_t[c][:, th * 512:(th + 1) * 512],
                    start=(c == 0), stop=(c == FC - 1))
            nc.vector.tensor_scalar(kT_t[fc][:, th * 512:(th + 1) * 512], ps[:],
                                    bk_sb[:, fc:fc + 1], None, Alu.add)

    prb = attn  # probs tiles come from the attn pool (small, consumed jc-wise)

    def emit_st_exp(hp, ih):
        i0 = ih * 512
        ptiles = [None] * KC
        for jc in range(KC):
            st2 = pst.tile([128, 1024], F32, tag="st", name=f"st{hp}_{ih}_{jc}")
            nc.tensor.matmul(
                st2[:, 0:512],
                lhsT=kT_t[hp][0:64, jc * 128:(jc + 1) * 128],
                rhs=qT_t[hp][0:64, i0:i0 + 512],
                start=True, stop=True, tile_position=(0, 0))
            nc.tensor.matmul(
                st2[:, 512:1024],
                lhsT=kT_t[hp][64:128, jc * 128:(jc + 1) * 128],
                rhs=qT_t[hp][64:128, i0:i0 + 512],
                start=True, stop=True, tile_position=(64, 0))
            pr = prb.tile([128, 1024], BF16, tag="pr", bufs=18,
                          name=f"pr{hp}_{ih}_{jc}")
            nc.scalar.activation(pr[:], st2[:], Act.Exp, scale=0.125)
            ptiles[jc] = pr
        return ptiles

    def emit_pv(hp, ih, ptiles):
        i0 = ih * 512
        oe = po.tile([128, POW], F32, tag="po", name=f"oe{hp}_{ih}")
        # jc-outer so each probs tile is consumed by its 8 matmuls and freed.
        # One start per 2KB zero region (it marks the whole region pending-
        # zero, so the other chains' first writes overwrite correctly with
        # start=False); explicit ordering deps keep the start matmul first.
        bank_start = {}
        for jc in range(KC):
            for ic in range(4):
                for hh in range(2):
                    h = 2 * hp + hh
                    ob = CHAIN_OFF[ic] + hh * (HD + 1)
                    bank = 0 if ic < 3 else 1
                    is_start = jc == 0 and bank not in bank_start
                    is_stop = jc == KC - 1 and (
                        (bank == 0 and ic == 2 and hh == 1) or (bank == 1 and hh == 1))
                    mm = nc.tensor.matmul(
                        oe[:, ob:ob + HD + 1],
                        lhsT=ptiles[jc][:, hh * 512 + ic * 128:hh * 512 + ic * 128 + 128],
                        rhs=v_t[jc][:, h * (HD + 1):(h + 1) * (HD + 1)],
                        start=is_start, stop=is_stop, skip_group_check=True)
                    if is_start:
                        bank_start[bank] = mm.ins
                    elif jc == 0:
                        tile.add_dep_helper(mm.ins, bank_start[bank], sync=False,
                                            reason="psum zero-region start first")
        for ic in range(4):
            trr = oe[:, TR_OFF[ic]:TR_OFF[ic] + 64].bitcast(BF16)
            od2 = attn.tile([128, 2 * HD], BF16, tag="od", bufs=4, name=f"od{hp}_{ih}_{ic}")
            for hh in range(2):
                ob = CHAIN_OFF[ic] + hh * (HD + 1)
                rec = attn.tile([128, 1], F32, tag="rec", bufs=4, name=f"rec{hp}_{ih}_{ic}_{hh}")
                nc.vector.reciprocal(rec[:], oe[:, ob + HD:ob + HD + 1])
                nc.vector.tensor_scalar(od2[:, hh * HD:(hh + 1) * HD],
                                        oe[:, ob:ob + HD], rec[:], None, Alu.mult)
            # single [128,128] transpose: out row r = od2[:, r] -> rows 0..63
            # are head h0's features, 64..127 head h1's = the oT pair layout
            nc.tensor.transpose(trr[:], od2[:], ident[:])
            nc.vector.tensor_copy(
                oT_t[hp][:, i0 + ic * 128:i0 + (ic + 1) * 128], trr[:])

    # ---- software-pipelined qk + attention ----
    # qk(0) and the first 16 score tiles are emitted BEFORE the V projection
    # so the ACT engine has exp work during the (PE-bound) V phase; their
    # probs buffer in the 16-slot prb tag until PV(0,0) can consume them.
    emit_qk(0)
    pts00 = emit_st_exp(0, 0)
    emit_v()
    emit_pv(0, 0, pts00)
    pts01 = emit_st_exp(0, 1)
    emit_qk(1)
    emit_pv(0, 1, pts01)
    pwo = None
    pf = None
    foA = []
    wo_t = []

    for hp in range(1, FC):
        pts = [emit_st_exp(hp, ih) for ih in range(T // 512)]
        if hp + 1 < FC:
            emit_qk(hp + 1)
        if hp == FC - 2:
            # last qk just emitted: free the input pools and run the FIRST
            # HALF of the output projection (head pairs 0..3, complete by
            # now) in the ACT-paced tail where the PE otherwise idles.
            stack_qk.close()
            stack_x.close()
            pp.close()
            pwo = ctx.enter_context(tc.tile_pool(name="pwo", bufs=1))
            pf = ctx.enter_context(tc.tile_pool(name="pf", bufs=2, space="PSUM"))
            wo_t = [pwo.tile([128, H], BF16, tag="woT", bufs=FC, name=f"wo{i}")
                    for i in range(FC)]
            for fc in range(FC):
                nc.sync.dma_start(wo_t[fc][:], woT[fc * 128:(fc + 1) * 128, :])
            for tcn in range(T // 128):
                fa = pwo.tile([128, H], F32, tag="foA", bufs=T // 128, name=f"foA{tcn}")
                foA.append(fa)
                for mh in range(H // 512):
                    psf = pf.tile([128, 512], F32, tag="pf", name=f"pfa{tcn}_{mh}")
                    for fc in range(FC // 2):
                        nc.tensor.matmul(
                            psf[:], lhsT=oT_t[fc][:, tcn * 128:(tcn + 1) * 128],
                            rhs=wo_t[fc][:, mh * 512:(mh + 1) * 512],
                            start=(fc == 0), stop=(fc == FC // 2 - 1))
                    nc.vector.tensor_copy(fa[:, mh * 512:(mh + 1) * 512], psf[:])
        for ih in range(T // 512):
            emit_pv(hp, ih, pts[ih])

    # ---- second half of the output projection + combine ----
    for tcn in range(T // 128):
        fo = pwo.tile([128, H], F32, tag="fo", bufs=2, name=f"fo{tcn}")
        for mh in range(H // 512):
            psf = pf.tile([128, 512], F32, tag="pf", name=f"pfb{tcn}_{mh}")
            for fc in range(FC // 2, FC):
                nc.tensor.matmul(
                    psf[:], lhsT=oT_t[fc][:, tcn * 128:(tcn + 1) * 128],
                    rhs=wo_t[fc][:, mh * 512:(mh + 1) * 512],
                    start=(fc == FC // 2), stop=(fc == FC - 1))
            nc.vector.tensor_add(fo[:, mh * 512:(mh + 1) * 512], psf[:],
                                 foA[tcn][:, mh * 512:(mh + 1) * 512])
        nc.sync.dma_start(out[tcn * 128:(tcn + 1) * 128, :], fo[:])


def _build():
    nc = bacc.Bacc("TRN2", target_bir_lowering=False, debug=False, enable_asserts=True)
    d = {}
    d["xT"] = nc.dram_tensor("xT", [H, TK], BF16, kind="ExternalInput").ap()
    d["wqT"] = nc.dram_tensor("wqT", [H, H], BF16, kind="ExternalInput").ap()
    d["wkT"] = nc.dram_tensor("wkT", [H, H], BF16, kind="ExternalInput").ap()
    d["wvT"] = nc.dram_tensor("wvT", [H, H], BF16, kind="ExternalInput").ap()
    d["woT"] = nc.dram_tensor("woT", [H, H], BF16, kind="ExternalInput").ap()
    d["bqc"] = nc.dram_tensor("bqc", [128, FC], F32, kind="ExternalInput").ap()
    d["bkc"] = nc.dram_tensor("bkc", [128, FC], F32, kind="ExternalInput").ap()
    d["out"] = nc.dram_tensor("out", [T, H], F32, kind="ExternalOutput").ap()
    with tile.TileContext(nc) as tc:
        with ExitStack() as ctx:
            _emit(ctx, tc, d)
    nc.compile()
    return nc


def get_nc():
    if "nc" not in _CACHE:
        _CACHE["nc"] = _build()
    return _CACHE["nc"]


def make_in_maps(inputs):
    x = np.asarray(inputs["hidden_states"], dtype=np.float32)
    wq = np.asarray(inputs["wq"], dtype=np.float32)
    wk = np.asarray(inputs["wk"], dtype=np.float32)
    wv = np.asarray(inputs["wv"], dtype=np.float32)
    wo = np.asarray(inputs["wo"], dtype=np.float32)
    bq = np.asarray(inputs["bq"], dtype=np.float32)
    bk = np.asarray(inputs["bk"], dtype=np.float32)
    bv = np.asarray(inputs["bv"], dtype=np.float32)

    wqT = np.ascontiguousarray(wq.T).astype(BFNP)
    wkT = np.ascontiguousarray(wk.T).astype(BFNP)
    wvT = np.ascontiguousarray(wv.T).astype(BFNP)
    woT = np.ascontiguousarray(wo.T).astype(BFNP)
    # feature-major bias chunks: partition p, col fc -> bias[fc*128 + p]
    bqc = np.ascontiguousarray(bq.reshape(FC, 128).T)
    bkc = np.ascontiguousarray(bk.reshape(FC, 128).T)

    in_maps = []
    for c in range(NCORES):
        b, hf = divmod(c, 2)
        xb = x[b]
        # roll so this core's query tokens are tokens [0:T); key order is
        # irrelevant to attention (softmax/PV sum over keys).
        rolled = np.concatenate([xb[hf * T:], xb[:hf * T]], axis=0) if hf else xb
        xT = np.ascontiguousarray(rolled.T).astype(BFNP)
        in_maps.append({
            "xT": xT, "wqT": wqT, "wkT": wkT, "wvT": wvT,
            "woT": woT, "bqc": bqc, "bkc": bkc,
        })
    return in_maps


def kernel(**inputs):
    nc = get_nc()
    in_maps = make_in_maps(inputs)
    res = run_bass_kernel_spmd(nc, in_maps, core_ids=list(range(NCORES)))
    bo = np.asarray(inputs["bo"], dtype=np.float32)
    bv = np.asarray(inputs["bv"], dtype=np.float32)
    wo = np.asarray(inputs["wo"], dtype=np.float32)
    out = np.empty((B, S, H), dtype=np.float32)
    for c in range(NCORES):
        b, hf = divmod(c, 2)
        out[b, hf * T:(hf + 1) * T, :] = res.results[c]["out"]
    # bo is linear in the output; bv passes through attention additively
    # (softmax rows sum to 1), so both fold into one exact fp32 correction.
    out += (bo + bv @ wo.T)[None, None, :]
    return out


# revision 26
# speedup vs baseline: 1.0053x; 1.0053x over previous
"""Multi-head attention (B=4, S=2048, H=1024, 16 heads) on 8 Trainium2 NeuronCores.

Sharding: data-parallel over (batch, seq-half) -> 8 fully independent shards, no
collectives. Each core computes attention for 1024 query tokens of one batch
element; K/V are computed redundantly for the full 2048-token sequence of that
batch (+25% projection flops buys zero cross-core communication).

Per-core scheme (feature-major "transposed" layout), software-pipelined so the
ACT engine (exp) starts while the PE is still doing projections:

  emit order: V projection -> [qk(0); for hp: ST/exp(hp); qk(hp+1); PV(hp)]
              -> output projection

  - qT/kT feature-major [H, tokens]; each 128-row chunk = one head pair
  - scores transposed S_T[j,i]; both heads of a pair write ONE [128,1024]
    PSUM tile ([h0 512i | h1 512i]) via tile_position row packing -> the
    paired matmuls issue back-to-back and overlap on the PE (K=64 solo
    streams at half rate; pairing measured 143ns vs 492ns per mm)
  - softmax: one exp per [128,1024] tile on ACT ((N+352)/1.2 ns); no max
    subtraction needed (|s|/8 < 8 << 88); denominators ride along as a 65th
    ones-column of v in the PV matmul
  - PV: stationary = probs chunk, moving = v_ext; all 8 accumulation chains
    (4 ic x 2 heads) of an (hp,ih) live in ONE [128,776] PSUM tile: cols
    0..519 hold the 8 o_ext chains, cols 520..775 take the PE-transposed
    (feature-major) outputs via a bf16 bitcast view -> probs tiles are
    consumed jc-at-a-time (prb pool can be tiny -> SBUF fits the overlap)
  - divide by denominator: reciprocal + per-partition tensor_scalar on DVE
  - output projection accumulates all 16 heads in PSUM (contraction over
    feature chunks = head pairs)
  - biases: bq/bk per-partition adds on the PSUM->SBUF copy (DVE); bv and bo
    folded into one exact host-side fp32 correction (+ bo + bv @ wo.T --
    softmax rows sum to 1, so a v-bias passes through attention additively).
"""

import numpy as np
import ml_dtypes
from contextlib import ExitStack

import concourse.tile as tile
from concourse import bacc, mybir
from concourse.bass_utils import run_bass_kernel_spmd
from concourse.masks import make_identity

B, S, H, NH, HD = 4, 2048, 1024, 16, 64
T = 1024          # query tokens per core
TK = 2048         # key tokens per core
NCORES = 8
FC = H // 128     # 8 feature chunks (= head pairs)
KC = TK // 128    # 16 key-token chunks
F32 = mybir.dt.float32
BF16 = mybir.dt.bfloat16
BFNP = ml_dtypes.bfloat16
Act = mybir.ActivationFunctionType
Alu = mybir.AluOpType

# po tile layout (f32 cols, [128,1024] = 2 PSUM banks; no accumulation chain
# crosses a 2KB zero-region boundary):
#   bank0: chains ic0/ic1/ic2 at 0/130/260 (65 cols per head), tr0 at 390
#   bank1: chain ic3 at 512, tr1/tr2/tr3 at 642/706/770
CHAIN_OFF = [0, 130, 260, 512]
TR_OFF = [390, 642, 706, 770]
POW = 1024

_CACHE = {}


def _emit(ctx: ExitStack, tc, d):
    nc = tc.nc
    xT, wqT, wkT, wvT = d["xT"], d["wqT"], d["wkT"], d["wvT"]
    woT, bqc, bkc, out = d["woT"], d["bqc"], d["bkc"], d["out"]

    pers = ctx.enter_context(tc.tile_pool(name="pers", bufs=1))
    ident = pers.tile([128, 128], BF16, tag="ident")
    make_identity(nc, ident[:])

    kT_t = [pers.tile([128, TK], BF16, tag="kT", bufs=FC, name=f"kT{i}") for i in range(FC)]
    qT_t = [pers.tile([128, T], BF16, tag="qT", bufs=FC, name=f"qT{i}") for i in range(FC)]
    v_t = [pers.tile([128, NH * (HD + 1)], BF16, tag="vsb", bufs=KC, name=f"v{i}") for i in range(KC)]
    oT_t = [pers.tile([128, T], BF16, tag="oT", bufs=FC, name=f"oT{i}") for i in range(FC)]
    bq_sb = pers.tile([128, FC], F32, tag="bq")
    bk_sb = pers.tile([128, FC], F32, tag="bk")
    nc.sync.dma_start(bq_sb[:], bqc[:, :])
    nc.sync.dma_start(bk_sb[:], bkc[:, :])

    # attention working pools, allocated below the ph1 input pools so their
    # slots don't depend on ph1 frees (enables ph1/attention overlap)
    attn = ctx.enter_context(tc.tile_pool(name="attn", bufs=1))
    # PSUM stack: pst(4) + po(2) live for the whole attention; ppool(2) on
    # top, released after the last qk chain; pf(2) then reuses its banks.
    pst = ctx.enter_context(tc.tile_pool(name="pst", bufs=2, space="PSUM"))
    po = ctx.enter_context(tc.tile_pool(name="po", bufs=1, space="PSUM"))

    stack_x = ExitStack()
    stack_qk = ExitStack()
    stack_v = ExitStack()

    pin = stack_x.enter_context(tc.tile_pool(name="pin", bufs=1))
    x_t = [pin.tile([128, TK], BF16, tag="xin", bufs=FC, name=f"x{i}") for i in range(FC)]
    # wq/wk staged in column halves: 8 slots of [128,512] per tensor; the
    # second-half tiles reuse the slots once head pairs 0..3 are projected.
    pqk = stack_qk.enter_context(tc.tile_pool(name="pqk", bufs=1))
    wq_t = {}
    wk_t = {}
    for half in range(2):
        for i in range(FC):
            wq_t[half, i] = pqk.tile([128, 512], BF16, tag="wqin", bufs=FC,
                                     name=f"wq{half}_{i}")
            wk_t[half, i] = pqk.tile([128, 512], BF16, tag="wkin", bufs=FC,
                                     name=f"wk{half}_{i}")
    pv_in = stack_v.enter_context(tc.tile_pool(name="pvin", bufs=1))
    wv_t = [pv_in.tile([128, H], BF16, tag="wvin", bufs=FC, name=f"wv{i}") for i in range(FC)]

    for fc in range(FC):
        # column-split so the first v chains unblock after half the bytes
        nc.sync.dma_start(x_t[fc][:, 0:1024], xT[fc * 128:(fc + 1) * 128, 0:1024])
        nc.sync.dma_start(wv_t[fc][:, 0:512], wvT[fc * 128:(fc + 1) * 128, 0:512])
    for fc in range(FC):
        nc.sync.dma_start(x_t[fc][:, 1024:2048], xT[fc * 128:(fc + 1) * 128, 1024:2048])
        nc.sync.dma_start(wv_t[fc][:, 512:1024], wvT[fc * 128:(fc + 1) * 128, 512:1024])
    for half in range(2):
        for fc in range(FC):
            c0 = half * 512
            nc.sync.dma_start(wq_t[half, fc][:], wqT[fc * 128:(fc + 1) * 128, c0:c0 + 512])
            nc.sync.dma_start(wk_t[half, fc][:], wkT[fc * 128:(fc + 1) * 128, c0:c0 + 512])
    pp = ExitStack()
    ppool = pp.enter_context(tc.tile_pool(name="ppool", bufs=2, space="PSUM"))

    # ---- V projection, token-major ----
    # bv is NOT applied on device: softmax rows sum to 1, so a v-bias passes
    # through attention additively and is folded into the host-side output
    # correction (bv @ wo.T), exactly.
    def emit_v():
        for kc in range(KC):
            nc.gpsimd.memset(v_t[kc][:], 1.0)  # ones cols survive at 65*h+64
            for mh in range(H // 512):
                ps = ppool.tile([128, 512], F32, tag="pp", name=f"psv{kc}_{mh}")
                for c in range(FC):
                    nc.tensor.matmul(
                        ps[:], lhsT=x_t[c][:, kc * 128:(kc + 1) * 128],
                        rhs=wv_t[c][:, mh * 512:(mh + 1) * 512],
                        start=(c == 0), stop=(c == FC - 1))
                # one strided copy scatters the 8 head slices (65-col pitch);
                # runs on ACT, which is otherwise idle during the V phase
                dst = v_t[kc][:, mh * 8 * (HD + 1):(mh + 1) * 8 * (HD + 1)]
                dst = dst.rearrange("p (h c) -> p h c", h=8)[:, :, 0:HD]
                src = ps[:, 0:512].rearrange("p (h c) -> p h c", h=8)
                nc.scalar.copy(dst, src)
        stack_v.close()

    def emit_qk(fc):
        half, cb = fc // 4, (fc % 4) * 128
        for th in range(T // 512):
            ps = ppool.tile([128, 512], F32, tag="pp", name=f"psq{fc}_{th}")
            for c in range(FC):
                nc.tensor.matmul(
                    ps[:], lhsT=wq_t[half, c][:, cb:cb + 128],
                    rhs=x_t[c][:, th * 512:(th + 1) * 512],
                    start=(c == 0), stop=(c == FC - 1))
            nc.vector.tensor_scalar(qT_t[fc][:, th * 512:(th + 1) * 512], ps[:],
                                    bq_sb[:, fc:fc + 1], None, Alu.add)
        for th in range(TK // 512):
            ps = ppool.tile([128, 512], F32, tag="pp", name=f"psk{fc}_{th}")
            for c in range(FC):
                nc.tensor.matmul(
                    ps[:], lhsT=wk_t[half, c][:, cb:cb + 128],
                    rhs=x_t[c][:, th * 512:(th + 1) * 512],
                    start=(c == 0), stop=(c == FC - 1))
            nc.vector.tensor_scalar(kT_t[fc][:, th * 512:(th + 1) * 512], ps[:],
                                    bk_sb[:, fc:fc + 1], None, Alu.add)

    prb = attn  # probs tiles come from the attn pool (small, consumed jc-wise)

    def emit_st_exp(hp, ih):
        i0 = ih * 512
        ptiles = [None] * KC
        for jc in range(KC):
            st2 = pst.tile([128, 1024], F32, tag="st", name=f"st{hp}_{ih}_{jc}")
            nc.tensor.matmul(
                st2[:, 0:512],
                lhsT=kT_t[hp][0:64, jc * 128:(jc + 1) * 128],
                rhs=qT_t[hp][0:64, i0:i0 + 512],
                start=True, stop=True, tile_position=(0, 0))
            nc.tensor.matmul(
                st2[:, 512:1024],
                lhsT=kT_t[hp][64:128, jc * 128:(jc + 1) * 128],
                rhs=qT_t[hp][64:128, i0:i0 + 512],
                start=True, stop=True, tile_position=(64, 0))
            pr = prb.tile([128, 1024], BF16, tag="pr", bufs=18,
                          name=f"pr{hp}_{ih}_{jc}")
            nc.scalar.activation(pr[:], st2[:], Act.Exp, scale=0.125)
            ptiles[jc] = pr
        return ptiles

    def emit_pv(hp, ih, ptiles):
        i0 = ih * 512
        oe = po.tile([128, POW], F32, tag="po", name=f"oe{hp}_{ih}")
        # jc-outer so each probs tile is consumed by its 8 matmuls and freed.
        # One start per 2KB zero region (it marks the whole region pending-
        # zero, so the other chains' first writes overwrite correctly with
        # start=False); explicit ordering deps keep the start matmul first.
        bank_start = {}
        for jc in range(KC):
            for ic in range(4):
                for hh in range(2):
                    h = 2 * hp + hh
                    ob = CHAIN_OFF[ic] + hh * (HD + 1)
                    bank = 0 if ic < 3 else 1
                    is_start = jc == 0 and bank not in bank_start
                    is_stop = jc == KC - 1 and (
                        (bank == 0 and ic == 2 and hh == 1) or (bank == 1 and hh == 1))
                    mm = nc.tensor.matmul(
                        oe[:, ob:ob + HD + 1],
                        lhsT=ptiles[jc][:, hh * 512 + ic * 128:hh * 512 + ic * 128 + 128],
                        rhs=v_t[jc][:, h * (HD + 1):(h + 1) * (HD + 1)],
                        start=is_start, stop=is_stop, skip_group_check=True)
                    if is_start:
                        bank_start[bank] = mm.ins
                    elif jc == 0:
                        tile.add_dep_helper(mm.ins, bank_start[bank], sync=False,
                                            reason="psum zero-region start first")
        for ic in range(4):
            trr = oe[:, TR_OFF[ic]:TR_OFF[ic] + 64].bitcast(BF16)
            od2 = attn.tile([128, 2 * HD], BF16, tag="od", bufs=4, name=f"od{hp}_{ih}_{ic}")
            for hh in range(2):
                ob = CHAIN_OFF[ic] + hh * (HD + 1)
                rec = attn.tile([128, 1], F32, tag="rec", bufs=4, name=f"rec{hp}_{ih}_{ic}_{hh}")
                nc.vector.reciprocal(rec[:], oe[:, ob + HD:ob + HD + 1])
                nc.vector.tensor_scalar(od2[:, hh * HD:(hh + 1) * HD],
                                        oe[:, ob:ob + HD], rec[:], None, Alu.mult)
            # single [128,128] transpose: out row r = od2[:, r] -> rows 0..63
            # are head h0's features, 64..127 head h1's = the oT pair layout
            nc.tensor.transpose(trr[:], od2[:], ident[:])
            nc.vector.tensor_copy(
                oT_t[hp][:, i0 + ic * 128:i0 + (ic + 1) * 128], trr[:])

    # ---- software-pipelined qk + attention ----
    # qk(0) and the first 16 score tiles are emitted BEFORE the V projection
    # so the ACT engine has exp work during the (PE-bound) V phase; their
    # probs buffer in the 16-slot prb tag until PV(0,0) can consume them.
    emit_qk(0)
    pts00 = emit_st_exp(0, 0)
    emit_v()
    emit_pv(0, 0, pts00)
    pts01 = emit_st_exp(0, 1)
    emit_qk(1)
    emit_pv(0, 1, pts01)
    pwo = None
    pf = None
    foA = []
    wo_t = []

    for hp in range(1, FC):
        pts = [emit_st_exp(hp, ih) for ih in range(T // 512)]
        if hp + 1 < FC:
            emit_qk(hp + 1)
        if hp == FC - 2:
            # last qk just emitted: free the input pools and run the FIRST
            # HALF of the output projection (head pairs 0..3, complete by
            # now) in the ACT-paced tail where the PE otherwise idles.
            stack_qk.close()
            stack_x.close()
            pp.close()
            pwo = ctx.enter_context(tc.tile_pool(name="pwo", bufs=1))
            pf = ctx.enter_context(tc.tile_pool(name="pf", bufs=2, space="PSUM"))
            wo_t = [pwo.tile([128, H], BF16, tag="woT", bufs=FC, name=f"wo{i}")
                    for i in range(FC)]
            for fc in range(FC):
                nc.sync.dma_start(wo_t[fc][:], woT[fc * 128:(fc + 1) * 128, :])
            for tcn in range(T // 128):
                fa = pwo.tile([128, H], F32, tag="foA", bufs=T // 128, name=f"foA{tcn}")
                foA.append(fa)
                for mh in range(H // 512):
                    psf = pf.tile([128, 512], F32, tag="pf", name=f"pfa{tcn}_{mh}")
                    for fc in range(FC // 2):
                        nc.tensor.matmul(
                            psf[:], lhsT=oT_t[fc][:, tcn * 128:(tcn + 1) * 128],
                            rhs=wo_t[fc][:, mh * 512:(mh + 1) * 512],
                            start=(fc == 0), stop=(fc == FC // 2 - 1))
                    nc.vector.tensor_copy(fa[:, mh * 512:(mh + 1) * 512], psf[:])
        if hp == FC - 1:
            # head pairs 4..6 of the projection fill the PE idle under the
            # last head pair's exp window; accumulated in-place into foA.
            for tcn in range(T // 128):
                for mh in range(H // 512):
                    psf = pf.tile([128, 512], F32, tag="pf", name=f"pfm{tcn}_{mh}")
                    for fc in range(FC // 2, FC - 1):
                        nc.tensor.matmul(
                            psf[:], lhsT=oT_t[fc][:, tcn * 128:(tcn + 1) * 128],
                            rhs=wo_t[fc][:, mh * 512:(mh + 1) * 512],
                            start=(fc == FC // 2), stop=(fc == FC - 2))
                    nc.vector.tensor_add(foA[tcn][:, mh * 512:(mh + 1) * 512], psf[:],
                                         foA[tcn][:, mh * 512:(mh + 1) * 512])
        for ih in range(T // 512):
            emit_pv(hp, ih, pts[ih])

    # ---- last projection slice (head pair 7) + combine ----
    for tcn in range(T // 128):
        fo = pwo.tile([128, H], F32, tag="fo", bufs=2, name=f"fo{tcn}")
        for mh in range(H // 512):
            psf = pf.tile([128, 512], F32, tag="pf", name=f"pfb{tcn}_{mh}")
            nc.tensor.matmul(
                psf[:], lhsT=oT_t[FC - 1][:, tcn * 128:(tcn + 1) * 128],
                rhs=wo_t[FC - 1][:, mh * 512:(mh + 1) * 512],
                start=True, stop=True)
            nc.vector.tensor_add(fo[:, mh * 512:(mh + 1) * 512], psf[:],
                                 foA[tcn][:, mh * 512:(mh + 1) * 512])
        nc.sync.dma_start(out[tcn * 128:(tcn + 1) * 128, :], fo[:])


def _build():
    nc = bacc.Bacc("TRN2", target_bir_lowering=False, debug=False, enable_asserts=True)
    d = {}
    d["xT"] = nc.dram_tensor("xT", [H, TK], BF16, kind="ExternalInput").ap()
    d["wqT"] = nc.dram_tensor("wqT", [H, H], BF16, kind="ExternalInput").ap()
    d["wkT"] = nc.dram_tensor("wkT", [H, H], BF16, kind="ExternalInput").ap()
    d["wvT"] = nc.dram_tensor("wvT", [H, H], BF16, kind="ExternalInput").ap()
    d["woT"] = nc.dram_tensor("woT", [H, H], BF16, kind="ExternalInput").ap()
    d["bqc"] = nc.dram_tensor("bqc", [128, FC], F32, kind="ExternalInput").ap()
    d["bkc"] = nc.dram_tensor("bkc", [128, FC], F32, kind="ExternalInput").ap()
    d["out"] = nc.dram_tensor("out", [T, H], F32, kind="ExternalOutput").ap()
    with tile.TileContext(nc) as tc:
        with ExitStack() as ctx:
            _emit(ctx, tc, d)
    nc.compile()
    return nc


def get_nc():
    if "nc" not in _CACHE:
        _CACHE["nc"] = _build()
    return _CACHE["nc"]


def make_in_maps(inputs):
    x = np.asarray(inputs["hidden_states"], dtype=np.float32)
    wq = np.asarray(inputs["wq"], dtype=np.float32)
    wk = np.asarray(inputs["wk"], dtype=np.float32)
    wv = np.asarray(inputs["wv"], dtype=np.float32)
    wo = np.asarray(inputs["wo"], dtype=np.float32)
    bq = np.asarray(inputs["bq"], dtype=np.float32)
    bk = np.asarray(inputs["bk"], dtype=np.float32)
    bv = np.asarray(inputs["bv"], dtype=np.float32)

    wqT = np.ascontiguousarray(wq.T).astype(BFNP)
    wkT = np.ascontiguousarray(wk.T).astype(BFNP)
    wvT = np.ascontiguousarray(wv.T).astype(BFNP)
    woT = np.ascontiguousarray(wo.T).astype(BFNP)
    # feature-major bias chunks: partition p, col fc -> bias[fc*128 + p]
    bqc = np.ascontiguousarray(bq.reshape(FC, 128).T)
    bkc = np.ascontiguousarray(bk.reshape(FC, 128).T)

    in_maps = []
    for c in range(NCORES):
        b, hf = divmod(c, 2)
        xb = x[b]
        # roll so this core's query tokens are tokens [0:T); key order is
        # irrelevant to attention (softmax/PV sum over keys).
        rolled = np.concatenate([xb[hf * T:], xb[:hf * T]], axis=0) if hf else xb
        xT = np.ascontiguousarray(rolled.T).astype(BFNP)
        in_maps.append({
            "xT": xT, "wqT": wqT, "wkT": wkT, "wvT": wvT,
            "woT": woT, "bqc": bqc, "bkc": bkc,
        })
    return in_maps


def kernel(**inputs):
    nc = get_nc()
    in_maps = make_in_maps(inputs)
    res = run_bass_kernel_spmd(nc, in_maps, core_ids=list(range(NCORES)))
    bo = np.asarray(inputs["bo"], dtype=np.float32)
    bv = np.asarray(inputs["bv"], dtype=np.float32)
    wo = np.asarray(inputs["wo"], dtype=np.float32)
    out = np.empty((B, S, H), dtype=np.float32)
    for c in range(NCORES):
        b, hf = divmod(c, 2)
        out[b, hf * T:(hf + 1) * T, :] = res.results[c]["out"]
    # bo is linear in the output; bv passes through attention additively
    # (softmax rows sum to 1), so both fold into one exact fp32 correction.
    out += (bo + bv @ wo.T)[None, None, :]
    return out
